# revision 4
# baseline (speedup 1.0000x reference)
"""Trainium2 Bass kernel for nn_Classifier (segment_reduce).

Computation (reference):
    local  = relu(x @ W1.T)            # [T, 50] @ [50, 400] -> [T, 400]
    feat   = mean over windows of J=24 # [T//24, 400]
    logits = feat @ W2.T               # [T//24, 400] @ [400, 10]

Strategy: pure data parallel over 8 NeuronCores (x sharded along T).
Per core (T_c = 98304 rows = 4096 windows), per supergroup G (3072 xq
cols = 6144 rows = 256 windows, 128 per shard-half):
  - Host packs the x shard TRANSPOSED + fp8(e3m4, x2 prescale) into
    xq [100, 49152]: rows 0-49 hold x_shard[:49152].T, rows 50-99 hold
    x_shard[49152:].T. fp8 quarters the upload volume vs f32 (rel err
    ~6e-3 incl. quantization, vs the 2e-2 gate); the x2 prescale (folded
    back via w2tp) halves the subnormal-region quantization step.
    On-chip the halves sit at partitions 0-49 / 64-113 so the two
    matmul1 tiles row-tile the PE array (tile_position (0,0)/(64,0))
    for 2x concurrent matmuls; the contraction dim (n=50) sits on
    partitions so matmul1 needs no on-device transpose.
  - matmul1: stationary = xq 128-col fp8 tile, moving = W1.T [50, 400]
    bf16 -> psum pair [128t, 2x512] fp32.
  - relu evacuation psum->sbuf bf16 alternates ScalarE / VectorE —
    the throughput-limiting stage (PSUM-sourced ops run 1x on both).
  - pooling on the PE: strip boundaries align with every 6th tile
    (768 rows = 32 windows exactly), so 6 shared 0/1 stationaries
    produce feat in NATURAL window order across 4 col-strips. Each
    pooling matmul is split into two concurrent 64-contraction tiles
    (rows 0-63 / 64-127): in-order dispatch serializes same-position
    tiles, so the lower tile of pair i overlaps the upper tile of
    pair i+1 and pooling costs half the PE cycles.
  - feat -> sbuf (ScalarE), k-transposed via ONE xbar DMA transpose
    [128, 1024] -> [128, 8, 128] (no PE transposes, no perm scramble),
    then matmul2 accumulates logits over 4 k-chunks per half.
  - The tail is lagged one supergroup so the PE never stalls on it.
"""

import sys

sys.path.insert(0, "/opt/trn_rl_repo")

import numpy as np
import ml_dtypes

import bass_rust
import concourse.bass as bass
import concourse.mybir as mybir
import concourse.tile as tile
from concourse.bass_utils import run_bass_kernel_spmd
from concourse.tile import TileContext
from concourse.vector_clock import ScopedClock

# ---------------------------------------------------------------------------
# Wait-count legalization (monkeypatch).
#
# This walrus build accepts at most 1 sync-wait per instruction (2 for
# EventSemaphore), but Tile's scheduler and tail drain can attach more,
# failing codegen with "Too many sync wait commands". Spread excess waits
# onto same-engine NOPs inserted immediately before the instruction.
# ---------------------------------------------------------------------------

_orig_add = TileContext._add_instruction


def _wait_cap(inst):
    return 2 if type(inst).__name__ == "InstEventSemaphore" else 1


def _patched_add_instruction(self, inst):
    si = inst.sync_info
    cap = _wait_cap(inst)
    if (
        si is not None
        and si.on_wait
        and len(si.on_wait) > cap
        and inst.engine != mybir.EngineType.Unassigned
    ):
        waits = list(si.on_wait)
        for w in waits[:-cap]:
            nop = bass_rust.InstNoOp(
                name=f"I-waitfix-{self.nc.next_id()}",
                opcode="NoOp",
                engine=inst.engine,
                ins=[],
                outs=[],
            )
            nop.sync_info = mybir.SyncInfo(on_wait=[w], on_update=[])
            _orig_add(self, nop)
        inst.sync_info = mybir.SyncInfo(
            on_wait=waits[-cap:], on_update=list(si.on_update or [])
        )
    _orig_add(self, inst)


def _patched_drain_and_barrier(self, tick_clock, wait_clock):
    nc = self.nc
    drain_inst = nc.sync.drain()
    wait_clock.add_sem_waits(
        drain_inst.ins, ScopedClock({None: tick_clock.global_clock})
    )
    mi = drain_inst.ins
    si = mi.sync_info
    waits = list(si.on_wait) if (si and si.on_wait) else []
    if len(waits) > 1:
        mi.sync_info = mybir.SyncInfo(
            on_wait=[waits[-1]], on_update=list(si.on_update or [])
        )
        for w in waits[:-1]:
            nop = nc.sync.nop()
            nop.ins.sync_info = mybir.SyncInfo(on_wait=[w], on_update=[])

    nc.all_engine_barrier()
    assert self.sems is not None
    popped = nc._tile_sem_poison_stack.pop()
    assert popped is self._sem_poison
    nc.clear_and_free_semaphores(list(self.sems.allocated().values()))
    nc.all_engine_barrier()


TileContext._add_instruction = _patched_add_instruction
TileContext._drain_and_barrier = _patched_drain_and_barrier

# ---------------------------------------------------------------------------
# Problem constants (hardcoded per the harness contract)
# ---------------------------------------------------------------------------

J = 24
T, N, K, C = 786432, 50, 400, 10
NCORES = 8
TC = T // NCORES          # 98304 rows per core
H = TC // 2               # 49152 cols per half in xq
B_CORE = TC // J          # 4096 windows per core
NG = 16                   # supergroup iterations per repeat
CHUNK = 24 * 128          # 3072 xq columns per supergroup
NP = 24                   # mm1 pairs (128-col x tiles) per supergroup
SX = 2.0                  # x prescale before fp8 quantization

BF16 = mybir.dt.bfloat16
F32 = mybir.dt.float32
FP8 = mybir.dt.float8e3
nbf = ml_dtypes.bfloat16
nf8 = ml_dtypes.float8_e3m4

POOL_SPLIT = False         # two concurrent 64-row pooling tiles per pair


def _build_pmats():
    """Six pooling stationaries P_j [128, 32] packed as [128, 192].
    Strip boundaries align with every 6th 128-row tile (768 rows = 32
    windows), so P_j depends only on j = tile_index % 6:
    P_j[tau, (128*j + tau) // 24] = 1."""
    pm = np.zeros((128, 192), np.float32)
    for j_ in range(6):
        for tau in range(128):
            pm[tau, 32 * j_ + (128 * j_ + tau) // 24] = 1.0
    return pm.astype(nbf)


def _build_w2tp(W2):
    """W2/(24*SX) arranged for matmul2 over 128-row k-chunks:
    w2tp[r, 10c+cc] = W2[cc, 128c+r]/(24*SX) (zero past k=400)."""
    w = np.zeros((128, 40), np.float32)
    for c in range(4):
        k0 = 128 * c
        kn = min(400, k0 + 128) - k0
        w[:kn, 10 * c : 10 * c + 10] = (
            W2.astype(np.float32).T[k0 : k0 + kn]
        ) / (J * SX)
    return w.astype(nbf)


def _build_nc(repeat: int = 1):
    """repeat>1 re-runs the whole computation in one NEFF — used by the
    test harness to measure device time differentially."""
    nc = bass.Bass()
    xq_d = nc.declare_dram_parameter("xq", [100, H], FP8, isOutput=False)
    w1t_d = nc.declare_dram_parameter("w1t", [50, 400], BF16, isOutput=False)
    w2tp_d = nc.declare_dram_parameter("w2tp", [128, 40], BF16, isOutput=False)
    pm_d = nc.declare_dram_parameter("pmats", [128, 192], BF16, isOutput=False)
    out_d = nc.declare_dram_parameter("logits", [B_CORE, 10], F32, isOutput=True)

    act = mybir.ActivationFunctionType

    with TileContext(nc) as tc:
        with (
            tc.tile_pool(name="consts", bufs=1) as cpool,
            tc.tile_pool(name="xchunks", bufs=3) as xpool,
            tc.tile_pool(name="relu", bufs=26) as rpool,
            tc.tile_pool(name="featsb", bufs=2) as fspool,
            tc.tile_pool(name="featT", bufs=2) as ftpool,
            tc.tile_pool(name="lsb", bufs=2) as lpool,
            # pairs bufs=3 (6 banks) decouples the evac engines from the
            # mm1 refill latency; the logits accumulator shares the pair
            # ring (25th alloc/sg) so everything fits in 8 banks.
            tc.tile_pool(name="mm1ps", bufs=3, space="PSUM") as mm1pool,
            tc.tile_pool(name="featps", bufs=2, space="PSUM") as featpool,
        ):
            # W1T staged at partition offsets 0 and 64 — the moving operand
            # must share the stationary's base partition (array row offset).
            w1t = cpool.tile([128, 400], BF16)
            w2tp = cpool.tile([128, 40], BF16)
            pmats = cpool.tile([128, 192], BF16)
            nc.sync.dma_start(out=w1t[0:50, :], in_=w1t_d[:])
            nc.sync.dma_start(out=w1t[64:114, :], in_=w1t_d[:])
            nc.sync.dma_start(out=w2tp[:], in_=w2tp_d[:])
            nc.sync.dma_start(out=pmats[:], in_=pm_d[:])

            def emit_tail_a(st):
                """feat psum -> sbuf (+pad memset) + xbar transpose."""
                feat_sb = fspool.tile([128, 2, 512], BF16, name="fsb")
                nc.gpsimd.memset(feat_sb[:, :, 400:512], 0.0)
                for hh in range(2):
                    nc.scalar.activation(
                        feat_sb[:, hh, 0:400], st["featps"][hh][:, 0:400], act.Relu
                    )
                featT = ftpool.tile([128, 8, 128], BF16, name="ftT")
                nc.sync.dma_start_transpose(
                    out=featT[:], in_=feat_sb[:].rearrange("p two k -> p (two k)")
                )
                st["featT"] = featT

            def emit_tail_b(st):
                """matmul2 over k-chunks + logits psum -> sbuf -> DRAM."""
                featT = st["featT"]
                lps = mm1pool.tile([128, 1024], F32, name="ps")
                for hh in range(2):
                    for c_ in range(4):
                        nc.tensor.matmul(
                            lps[:, 16 * hh : 16 * hh + 10],
                            featT[:, 4 * hh + c_, :],
                            w2tp[:, 10 * c_ : 10 * c_ + 10],
                            start=(c_ == 0),
                            stop=(c_ == 3),
                        )
                lsb = lpool.tile([128, 2, 10], F32, name="lsb")
                src = lps[:, 0:32].rearrange("p (two k) -> p two k", two=2)[
                    :, :, 0:10
                ]
                nc.vector.tensor_copy(out=lsb[:], in_=src)
                g = st["G"]
                dst = out_d[:].rearrange(
                    "(two w) c -> w two c", two=2
                )[128 * g : 128 * g + 128]
                nc.sync.dma_start(out=dst, in_=lsb[:])

            prev = None  # supergroup awaiting tail-a (feat evac + transpose)
            prev2 = None  # supergroup awaiting tail-b (mm2 + store)
            for G in [g for _ in range(repeat) for g in range(NG)]:
                xc = xpool.tile([128, CHUNK], FP8, name="xc")
                nc.sync.dma_start(
                    out=xc[0:50, :], in_=xq_d[0:50, G * CHUNK : (G + 1) * CHUNK]
                )
                nc.sync.dma_start(
                    out=xc[64:114, :], in_=xq_d[50:100, G * CHUNK : (G + 1) * CHUNK]
                )

                if prev is not None:
                    emit_tail_a(prev)

                # ---- Phase A: matmul1 + relu evacuation (24 pairs) ----
                pairs = []
                for i in range(NP):
                    tcol = i * 128
                    ps = mm1pool.tile([128, 1024], F32, name="ps")
                    for hh in range(2):
                        rb = 64 * hh
                        nc.tensor.matmul(
                            ps[:, 512 * hh : 512 * hh + 400],
                            xc[rb : rb + 50, tcol : tcol + 128],
                            w1t[rb : rb + 50, :],
                            start=True,
                            stop=True,
                        )
                    rl = rpool.tile([128, 2, 400], BF16, name="rl", bufs=26)
                    src = ps[:, :].rearrange("p (two k) -> p two k", two=2)[
                        :, :, 0:400
                    ]
                    # measured sustained: ACT 650ns vs DVE 1040ns per pair
                    # (PSUM-sourced) -> 14:10 split (ACT also owns tail-a)
                    if i % 12 in (0, 1, 3, 5, 7, 9, 10):
                        nc.scalar.activation(rl[:], src, act.Relu)
                    else:
                        nc.vector.tensor_scalar_max(rl[:], src, 0.0)
                    pairs.append(rl)

                # ---- Phase B: pooling matmuls (natural window order) ----
                featps = [
                    featpool.tile([128, 512], F32, name="featps") for _ in range(2)
                ]
                for i in range(NP):
                    s, j_ = i // 6, i % 6
                    rl = pairs[i]
                    if POOL_SPLIT:
                        # Upper tiles (rows 0-63) serialize per strip; the
                        # lower tile of pair i dispatches after the upper
                        # tile of pair i (in-order), so it runs alongside
                        # the NEXT upper tile — halving pooling wall time.
                        # start=True only on the first upper write; the
                        # lower adds always land after it has completed.
                        for hh in range(2):
                            nc.tensor.matmul(
                                featps[hh][32 * s : 32 * s + 32, 0:400],
                                pmats[0:64, 32 * j_ : 32 * j_ + 32],
                                rl[0:64, hh, :],
                                start=(j_ == 0),
                                stop=False,
                                tile_position=(0, 32 * s),
                            )
                        for hh in range(2):
                            nc.tensor.matmul(
                                featps[hh][32 * s : 32 * s + 32, 0:400],
                                pmats[64:128, 32 * j_ : 32 * j_ + 32],
                                rl[64:128, hh, :],
                                start=False,
                                stop=(j_ == 5),
                                tile_position=(64, 32 * s),
                            )
                    else:
                        for hh in range(2):
                            nc.tensor.matmul(
                                featps[hh][32 * s : 32 * s + 32, 0:400],
                                pmats[:, 32 * j_ : 32 * j_ + 32],
                                rl[:, hh, :],
                                start=(j_ == 0),
                                stop=(j_ == 5),
                                tile_position=(0, 32 * s),
                            )

                if prev2 is not None:
                    emit_tail_b(prev2)
                prev2 = prev
                prev = {"G": G, "featps": featps}

            # drain the tail pipeline
            if prev is not None:
                emit_tail_a(prev)
            if prev2 is not None:
                emit_tail_b(prev2)
            emit_tail_b(prev)
    return nc


_NC = {}


def _get_nc(repeat: int = 1):
    if repeat not in _NC:
        _NC[repeat] = _build_nc(repeat)
    return _NC[repeat]


def prepare_in_maps(x: np.ndarray, W1: np.ndarray, W2: np.ndarray):
    assert x.shape == (T, N) and W1.shape == (K, N) and W2.shape == (C, K)

    w1t = np.ascontiguousarray(W1.T.astype(nbf))          # [50, 400]
    w2tp = _build_w2tp(W2)                                 # [128, 40]
    pmats = _build_pmats()

    xb = np.clip(x.astype(np.float32) * SX, -15.0, 15.0).astype(nf8)
    in_maps = []
    for c in range(NCORES):
        shard = xb[c * TC : (c + 1) * TC]                  # [98304, 50]
        xq = np.empty((100, H), nf8)
        xq[0:50] = shard[0:H].T
        xq[50:100] = shard[H:].T
        in_maps.append(
            {
                "xq": xq,
                "w1t": w1t,
                "w2tp": w2tp,
                "pmats": pmats,
            }
        )
    return in_maps


def kernel(x: np.ndarray, W1: np.ndarray, W2: np.ndarray) -> np.ndarray:
    in_maps = prepare_in_maps(x, W1, W2)
    nc = _get_nc()
    res = run_bass_kernel_spmd(nc, in_maps, core_ids=list(range(NCORES)))
    out = np.concatenate(
        [res.results[c]["logits"] for c in range(NCORES)], axis=0
    )
    return out.astype(np.float32)


# revision 14
# speedup vs baseline: 201.2079x; 201.2079x over previous
"""Trainium2 Bass kernel for nn_Classifier (segment_reduce).

Computation (reference):
    local  = relu(x @ W1.T)            # [T, 50] @ [50, 400] -> [T, 400]
    feat   = mean over windows of J=24 # [T//24, 400]
    logits = feat @ W2.T               # [T//24, 400] @ [400, 10]

Strategy: pure data parallel over 8 NeuronCores (x sharded along T).
Per core (T_c = 98304 rows = 4096 windows), per supergroup G (3072 xq
cols = 6144 rows = 256 windows, 128 per shard-half):
  - Host packs the x shard TRANSPOSED + fp8(e3m4, x2 prescale) into
    xq [100, 49152]: rows 0-49 hold x_shard[:49152].T, rows 50-99 hold
    x_shard[49152:].T. fp8 quarters the upload volume vs f32; the x2
    prescale (folded back via w2tp) halves the subnormal quantization
    step. On-chip the halves sit at partitions 0-49 / 64-113; the
    contraction dim (n=50) sits on partitions so matmul1 needs no
    on-device transpose.
  - matmul1: stationary = xq 128-col fp8 tile, moving = W1.T [50, 400]
    bf16 -> psum pair [128t, 2x512] fp32. PE instructions execute
    serially, so mm1 costs 400 cycles per half.
  - relu evacuation psum->sbuf fp8(e4m3) alternates ScalarE/VectorE —
    the throughput-limiting stage. Pairs are written two-to-a-tile
    ([128, 2, 2, 400]) so pooling can consume them DoubleRow-style.
  - pooling on the PE, hybrid: strips 0/2 (psum partition bases 0/64)
    use fp8 DoubleRow matmuls — one matmul computes P_j^T @ rl_j +
    P_{j+1}^T @ rl_{j+1} at 0.5 cycles/row — while strips 1/3 (bases
    32/96, unreachable by the 64-wide DoubleRow stationary whose
    tile_position col must be 0/64) use plain 32-wide matmuls.
    Pooling drops from 19200 to 12000 PE cycles per supergroup and
    feat lands in natural window order, one psum bank per half.
  - feat -> sbuf (ScalarE+VectorE), k-transposed via ONE xbar DMA
    transpose [128, 1024] -> [128, 8, 128], then matmul2 accumulates
    logits over 4 k-chunks per half.
  - The tail is lagged one supergroup so the PE never stalls on it.
"""

import sys

sys.path.insert(0, "/opt/trn_rl_repo")

import numpy as np
import ml_dtypes

import bass_rust
import concourse.bass as bass
import concourse.mybir as mybir
import concourse.tile as tile
from concourse.bass_utils import run_bass_kernel_spmd
from concourse.tile import TileContext
from concourse.vector_clock import ScopedClock

# ---------------------------------------------------------------------------
# Wait-count legalization (monkeypatch).
#
# This walrus build accepts at most 1 sync-wait per instruction (2 for
# EventSemaphore), but Tile's scheduler and tail drain can attach more,
# failing codegen with "Too many sync wait commands". Spread excess waits
# onto same-engine NOPs inserted immediately before the instruction.
# ---------------------------------------------------------------------------

_orig_add = TileContext._add_instruction


def _wait_cap(inst):
    return 2 if type(inst).__name__ == "InstEventSemaphore" else 1


def _patched_add_instruction(self, inst):
    si = inst.sync_info
    cap = _wait_cap(inst)
    if (
        si is not None
        and si.on_wait
        and len(si.on_wait) > cap
        and inst.engine != mybir.EngineType.Unassigned
    ):
        waits = list(si.on_wait)
        for w in waits[:-cap]:
            nop = bass_rust.InstNoOp(
                name=f"I-waitfix-{self.nc.next_id()}",
                opcode="NoOp",
                engine=inst.engine,
                ins=[],
                outs=[],
            )
            nop.sync_info = mybir.SyncInfo(on_wait=[w], on_update=[])
            _orig_add(self, nop)
        inst.sync_info = mybir.SyncInfo(
            on_wait=waits[-cap:], on_update=list(si.on_update or [])
        )
    _orig_add(self, inst)


def _patched_drain_and_barrier(self, tick_clock, wait_clock):
    nc = self.nc
    drain_inst = nc.sync.drain()
    wait_clock.add_sem_waits(
        drain_inst.ins, ScopedClock({None: tick_clock.global_clock})
    )
    mi = drain_inst.ins
    si = mi.sync_info
    waits = list(si.on_wait) if (si and si.on_wait) else []
    if len(waits) > 1:
        mi.sync_info = mybir.SyncInfo(
            on_wait=[waits[-1]], on_update=list(si.on_update or [])
        )
        for w in waits[:-1]:
            nop = nc.sync.nop()
            nop.ins.sync_info = mybir.SyncInfo(on_wait=[w], on_update=[])

    nc.all_engine_barrier()
    assert self.sems is not None
    popped = nc._tile_sem_poison_stack.pop()
    assert popped is self._sem_poison
    nc.clear_and_free_semaphores(list(self.sems.allocated().values()))
    nc.all_engine_barrier()


TileContext._add_instruction = _patched_add_instruction
TileContext._drain_and_barrier = _patched_drain_and_barrier

# ---------------------------------------------------------------------------
# Problem constants (hardcoded per the harness contract)
# ---------------------------------------------------------------------------

J = 24
T, N, K, C = 786432, 50, 400, 10
NCORES = 8
TC = T // NCORES          # 98304 rows per core
H = TC // 2               # 49152 cols per half in xq
B_CORE = TC // J          # 4096 windows per core
NG = 16                   # supergroup iterations per repeat
CHUNK = 24 * 128          # 3072 xq columns per supergroup
NP = 24                   # mm1 pairs (128-col x tiles) per supergroup
SX = 2.0                  # x prescale before fp8 quantization

BF16 = mybir.dt.bfloat16
F32 = mybir.dt.float32
FP8 = mybir.dt.float8e3
FP8E4 = mybir.dt.float8e4
nbf = ml_dtypes.bfloat16
nf8 = ml_dtypes.float8_e3m4

# Pooling via fp8e4 DoubleRow matmuls (0.5 cycles/row). Fallback False =
# serial bf16 pooling (slower PE, slightly better accuracy).
POOL_DR = False
RL_DT = FP8E4 if POOL_DR else BF16
DR = mybir.MatmulPerfMode.DoubleRow


def _build_pmats():
    """Pooling stationaries for DoubleRow pairs, packed [128, 192]:
    col 64*jp + 32*i + w holds P_{2jp+i}[tau, w] where
    P_j[tau, (128*j + tau) // 24] = 1 (strip boundaries align with every
    6th 128-row tile: 768 rows = 32 windows)."""
    pm = np.zeros((128, 192), np.float32)
    for j_ in range(6):
        jp, i = divmod(j_, 2)
        for tau in range(128):
            pm[tau, 64 * jp + 32 * i + (128 * j_ + tau) // 24] = 1.0
    return pm


def _build_w2tp(W2):
    """W2/(24*SX) arranged for matmul2 over 128-row k-chunks:
    w2tp[r, 10c+cc] = W2[cc, 128c+r]/(24*SX) (zero past k=400)."""
    w = np.zeros((128, 40), np.float32)
    for c in range(4):
        k0 = 128 * c
        kn = min(400, k0 + 128) - k0
        w[:kn, 10 * c : 10 * c + 10] = (
            W2.astype(np.float32).T[k0 : k0 + kn]
        ) / (J * SX)
    return w.astype(nbf)


def _build_nc(repeat: int = 1):
    """repeat>1 re-runs the whole computation in one NEFF — used by the
    test harness to measure device time differentially."""
    nc = bass.Bass()
    xq_d = nc.declare_dram_parameter("xq", [100, H], FP8, isOutput=False)
    w1t_d = nc.declare_dram_parameter("w1t", [50, 400], BF16, isOutput=False)
    w2tp_d = nc.declare_dram_parameter("w2tp", [128, 40], BF16, isOutput=False)
    pm_d = nc.declare_dram_parameter("pmats", [128, 192], RL_DT, isOutput=False)
    out_d = nc.declare_dram_parameter("logits", [B_CORE, 10], F32, isOutput=True)

    act = mybir.ActivationFunctionType

    with TileContext(nc) as tc:
        with (
            tc.tile_pool(name="consts", bufs=1) as cpool,
            tc.tile_pool(name="xchunks", bufs=3) as xpool,
            tc.tile_pool(name="relu", bufs=13) as rpool,
            tc.tile_pool(name="featsb", bufs=2) as fspool,
            tc.tile_pool(name="featT", bufs=2) as ftpool,
            tc.tile_pool(name="lsb", bufs=2) as lpool,
            # mm1 pair ring 3 bufs (6 banks) + feat 2 bufs x 1 bank = 8.
            # The logits accumulator shares the mm1 pair ring.
            tc.tile_pool(name="mm1ps", bufs=3, space="PSUM") as mm1pool,
            tc.tile_pool(name="featps", bufs=2, space="PSUM") as featpool,
        ):
            # W1T staged at partition offsets 0 and 64 — the moving operand
            # must share the stationary's base partition (array row offset).
            w1t = cpool.tile([128, 400], BF16)
            w2tp = cpool.tile([128, 40], BF16)
            pmats = cpool.tile([128, 192], RL_DT)
            nc.sync.dma_start(out=w1t[0:50, :], in_=w1t_d[:])
            nc.sync.dma_start(out=w1t[64:114, :], in_=w1t_d[:])
            nc.sync.dma_start(out=w2tp[:], in_=w2tp_d[:])
            nc.sync.dma_start(out=pmats[:], in_=pm_d[:])

            def emit_tail_a(st):
                """feat psum -> sbuf (+pad memset) + xbar transpose."""
                feat_sb = fspool.tile([128, 2, 512], BF16, name="fsb")
                nc.gpsimd.memset(feat_sb[:, :, 400:512], 0.0)
                nc.scalar.activation(
                    feat_sb[:, 0, 0:400], st["featps"][0][:, 0:400], act.Relu
                )
                nc.vector.tensor_scalar_max(
                    feat_sb[:, 1, 0:400], st["featps"][1][:, 0:400], 0.0
                )
                featT = ftpool.tile([128, 8, 128], BF16, name="ftT")
                nc.sync.dma_start_transpose(
                    out=featT[:],
                    in_=feat_sb[:].rearrange("p two k -> p (two k)"),
                )
                st["featT"] = featT

            def emit_tail_b(st):
                """matmul2 over k-chunks + logits psum -> sbuf -> DRAM."""
                featT = st["featT"]
                g = st["G"]
                lps = mm1pool.tile([128, 1024], F32, name="ps")
                for hh in range(2):
                    for c_ in range(4):
                        nc.tensor.matmul(
                            lps[:, 16 * hh : 16 * hh + 10],
                            featT[:, 4 * hh + c_, :],
                            w2tp[:, 10 * c_ : 10 * c_ + 10],
                            start=(c_ == 0),
                            stop=(c_ == 3),
                        )
                lsb = lpool.tile([128, 2, 10], F32, name="lsb")
                src = lps[:, 0:32].rearrange("p (two k) -> p two k", two=2)[
                    :, :, 0:10
                ]
                nc.vector.tensor_copy(out=lsb[:], in_=src)
                dst = out_d[:].rearrange(
                    "(two w) c -> w two c", two=2
                )[128 * g : 128 * g + 128]
                nc.sync.dma_start(out=dst, in_=lsb[:])

            prev = None  # supergroup awaiting tail-a (feat evac + transpose)
            prev2 = None  # supergroup awaiting tail-b (mm2 + store)
            for G in [g for _ in range(repeat) for g in range(NG)]:
                xc = xpool.tile([128, CHUNK], FP8, name="xc")
                nc.sync.dma_start(
                    out=xc[0:50, :], in_=xq_d[0:50, G * CHUNK : (G + 1) * CHUNK]
                )
                nc.sync.dma_start(
                    out=xc[64:114, :], in_=xq_d[50:100, G * CHUNK : (G + 1) * CHUNK]
                )

                if prev is not None:
                    emit_tail_a(prev)

                # ---- matmul1 + relu evacuation (24 pairs, written
                # two-to-a-tile) interleaved with lagged pooling so the
                # evac engines never drain at supergroup boundaries ----
                pairs = []  # 12 tiles [128, 2(pair member), 2(hh), 400]
                featps = [
                    featpool.tile([128, 512], F32, name="featps")
                    for _ in range(2)
                ]

                def emit_pool(ip):
                    """Pool pair-group ip (pairs 2ip, 2ip+1) into strip
                    s = ip//3. Strips 0/2 (psum bases 0/64) via fp8
                    DoubleRow; strips 1/3 (bases 32/96) via plain 32-wide
                    matmuls (DoubleRow's 64-wide stationary only sits at
                    array cols 0/64)."""
                    rl2 = pairs[ip]
                    s, jp = ip // 3, ip % 3
                    if POOL_DR and s % 2 == 0:
                        stat = pmats[:, 64 * jp : 64 * jp + 64].rearrange(
                            "p (two w) -> p two w", two=2
                        )
                        for hh in range(2):
                            nc.tensor.matmul(
                                featps[hh][32 * s : 32 * s + 32, 0:400],
                                stat,
                                rl2[:, :, hh, :],
                                start=(jp == 0),
                                stop=(jp == 2),
                                perf_mode=DR,
                                tile_position=(0, 32 * s),
                            )
                    else:
                        for im in range(2):
                            j_ = 2 * jp + im
                            for hh in range(2):
                                nc.tensor.matmul(
                                    featps[hh][32 * s : 32 * s + 32, 0:400],
                                    pmats[
                                        :,
                                        64 * jp + 32 * im : 64 * jp + 32 * im + 32,
                                    ],
                                    rl2[:, im, hh, :],
                                    start=(j_ == 0),
                                    stop=(j_ == 5),
                                    tile_position=(0, 32 * s),
                                )

                for ip in range(NP // 2):
                    rl2 = rpool.tile([128, 2, 2, 400], RL_DT, name="rl", bufs=13)
                    for im in range(2):
                        i = 2 * ip + im
                        tcol = i * 128
                        ps = mm1pool.tile([128, 1024], F32, name="ps")
                        for hh in range(2):
                            rb = 64 * hh
                            nc.tensor.matmul(
                                ps[:, 512 * hh : 512 * hh + 400],
                                xc[rb : rb + 50, tcol : tcol + 128],
                                w1t[rb : rb + 50, :],
                                start=True,
                                stop=True,
                            )
                        src = ps[:, :].rearrange("p (two k) -> p two k", two=2)[
                            :, :, 0:400
                        ]
                        # ACT ~850ns vs DVE ~960ns per pair (PSUM-sourced);
                        # ACT also owns half of tail-a -> 13:11 split
                        if im == 0 or ip == 5:
                            nc.scalar.activation(rl2[:, im, :, :], src, act.Relu)
                        else:
                            nc.vector.tensor_scalar_max(rl2[:, im, :, :], src, 0.0)
                    pairs.append(rl2)
                    if ip >= 2:
                        emit_pool(ip - 2)
                    if ip == 5 and prev2 is not None:
                        emit_tail_b(prev2)
                emit_pool(10)
                emit_pool(11)

                prev2 = prev
                prev = {"G": G, "featps": featps}

            # drain the tail pipeline
            if prev is not None:
                emit_tail_a(prev)
            if prev2 is not None:
                emit_tail_b(prev2)
            emit_tail_b(prev)
    return nc


_NC = {}


def _get_nc(repeat: int = 1):
    if repeat not in _NC:
        _NC[repeat] = _build_nc(repeat)
    return _NC[repeat]


def prepare_in_maps(x: np.ndarray, W1: np.ndarray, W2: np.ndarray):
    assert x.shape == (T, N) and W1.shape == (K, N) and W2.shape == (C, K)

    w1t = np.ascontiguousarray(W1.T.astype(nbf))          # [50, 400]
    w2tp = _build_w2tp(W2)                                 # [128, 40]
    pmats = _build_pmats().astype(mybir.dt.np(RL_DT))

    xb = np.clip(x.astype(np.float32) * SX, -15.0, 15.0).astype(nf8)
    in_maps = []
    for c in range(NCORES):
        shard = xb[c * TC : (c + 1) * TC]                  # [98304, 50]
        xq = np.empty((100, H), nf8)
        xq[0:50] = shard[0:H].T
        xq[50:100] = shard[H:].T
        in_maps.append(
            {
                "xq": xq,
                "w1t": w1t,
                "w2tp": w2tp,
                "pmats": pmats,
            }
        )
    return in_maps


def kernel(x: np.ndarray, W1: np.ndarray, W2: np.ndarray) -> np.ndarray:
    in_maps = prepare_in_maps(x, W1, W2)
    nc = _get_nc()
    res = run_bass_kernel_spmd(nc, in_maps, core_ids=list(range(NCORES)))
    out = np.concatenate(
        [res.results[c]["logits"] for c in range(NCORES)], axis=0
    )
    return out.astype(np.float32)


# revision 19
# speedup vs baseline: 483.3641x; 2.4023x over previous
"""Trainium2 Bass kernel for nn_Classifier (segment_reduce).

Computation (reference):
    local  = relu(x @ W1.T)            # [T, 50] @ [50, 400] -> [T, 400]
    feat   = mean over windows of J=24 # [T//24, 400]
    logits = feat @ W2.T               # [T//24, 400] @ [400, 10]

Strategy: pure data parallel over 8 NeuronCores (x sharded along T).
Per core (T_c = 98304 rows = 4096 windows), per supergroup G (3072 xq
cols = 6144 rows = 256 windows, 128 per shard-half):
  - Host packs the x shard TRANSPOSED + fp8(e3m4, x2 prescale) into
    xq [100, 49152]: rows 0-49 hold x_shard[:49152].T, rows 50-99 hold
    x_shard[49152:].T. fp8 quarters the upload volume vs f32; the x2
    prescale (folded back via w2tp) halves the subnormal quantization
    step. On-chip the halves sit at partitions 0-49 / 64-113; the
    contraction dim (n=50) sits on partitions so matmul1 needs no
    on-device transpose.
  - matmul1: stationary = xq 128-col fp8 tile, moving = W1.T [50, 400]
    bf16 -> psum pair [128t, 2x512] fp32. PE instructions execute
    serially, so mm1 costs 400 cycles per half.
  - relu evacuation psum->sbuf fp8(e4m3) alternates ScalarE/VectorE —
    the throughput-limiting stage. Pairs are written two-to-a-tile
    ([128, 2, 2, 400]) so pooling can consume them DoubleRow-style.
  - pooling on the PE, hybrid: strips 0/2 (psum partition bases 0/64)
    use fp8 DoubleRow matmuls — one matmul computes P_j^T @ rl_j +
    P_{j+1}^T @ rl_{j+1} at 0.5 cycles/row — while strips 1/3 (bases
    32/96, unreachable by the 64-wide DoubleRow stationary whose
    tile_position col must be 0/64) use plain 32-wide matmuls.
    Pooling drops from 19200 to 12000 PE cycles per supergroup and
    feat lands in natural window order, one psum bank per half.
  - feat -> sbuf (ScalarE+VectorE), k-transposed via ONE xbar DMA
    transpose [128, 1024] -> [128, 8, 128], then matmul2 accumulates
    logits over 4 k-chunks per half.
  - The tail is lagged one supergroup so the PE never stalls on it.
"""

import sys

sys.path.insert(0, "/opt/trn_rl_repo")

import numpy as np
import ml_dtypes

import bass_rust
import concourse.bass as bass
import concourse.mybir as mybir
import concourse.tile as tile
from concourse.bass_utils import run_bass_kernel_spmd
from concourse.tile import TileContext
from concourse.vector_clock import ScopedClock

# ---------------------------------------------------------------------------
# Wait-count legalization (monkeypatch).
#
# This walrus build accepts at most 1 sync-wait per instruction (2 for
# EventSemaphore), but Tile's scheduler and tail drain can attach more,
# failing codegen with "Too many sync wait commands". Spread excess waits
# onto same-engine NOPs inserted immediately before the instruction.
# ---------------------------------------------------------------------------

_orig_add = TileContext._add_instruction


def _wait_cap(inst):
    return 2 if type(inst).__name__ == "InstEventSemaphore" else 1


def _patched_add_instruction(self, inst):
    si = inst.sync_info
    cap = _wait_cap(inst)
    if (
        si is not None
        and si.on_wait
        and len(si.on_wait) > cap
        and inst.engine != mybir.EngineType.Unassigned
    ):
        waits = list(si.on_wait)
        for w in waits[:-cap]:
            nop = bass_rust.InstNoOp(
                name=f"I-waitfix-{self.nc.next_id()}",
                opcode="NoOp",
                engine=inst.engine,
                ins=[],
                outs=[],
            )
            nop.sync_info = mybir.SyncInfo(on_wait=[w], on_update=[])
            _orig_add(self, nop)
        inst.sync_info = mybir.SyncInfo(
            on_wait=waits[-cap:], on_update=list(si.on_update or [])
        )
    _orig_add(self, inst)


def _patched_drain_and_barrier(self, tick_clock, wait_clock):
    nc = self.nc
    drain_inst = nc.sync.drain()
    wait_clock.add_sem_waits(
        drain_inst.ins, ScopedClock({None: tick_clock.global_clock})
    )
    mi = drain_inst.ins
    si = mi.sync_info
    waits = list(si.on_wait) if (si and si.on_wait) else []
    if len(waits) > 1:
        mi.sync_info = mybir.SyncInfo(
            on_wait=[waits[-1]], on_update=list(si.on_update or [])
        )
        for w in waits[:-1]:
            nop = nc.sync.nop()
            nop.ins.sync_info = mybir.SyncInfo(on_wait=[w], on_update=[])

    nc.all_engine_barrier()
    assert self.sems is not None
    popped = nc._tile_sem_poison_stack.pop()
    assert popped is self._sem_poison
    nc.clear_and_free_semaphores(list(self.sems.allocated().values()))
    nc.all_engine_barrier()


TileContext._add_instruction = _patched_add_instruction
TileContext._drain_and_barrier = _patched_drain_and_barrier

# ---------------------------------------------------------------------------
# Problem constants (hardcoded per the harness contract)
# ---------------------------------------------------------------------------

J = 24
T, N, K, C = 786432, 50, 400, 10
NCORES = 8
TC = T // NCORES          # 98304 rows per core
H = TC // 2               # 49152 cols per half in xq
B_CORE = TC // J          # 4096 windows per core
NG = 16                   # supergroup iterations per repeat
CHUNK = 24 * 128          # 3072 xq columns per supergroup
NP = 24                   # mm1 pairs (128-col x tiles) per supergroup
SX = 2.0                  # x prescale before fp8 quantization

BF16 = mybir.dt.bfloat16
F32 = mybir.dt.float32
FP8 = mybir.dt.float8e3
FP8E4 = mybir.dt.float8e4
nbf = ml_dtypes.bfloat16
nf8 = ml_dtypes.float8_e3m4

# Pooling via fp8e4 DoubleRow matmuls (0.5 cycles/row) through the single
# legal DR destination (psum bank 0, partition base 0): one [128, 512]
# accumulator pools one (64-window strip, half) at a time. Fallback False =
# serial bf16 pooling (slower PE, slightly better accuracy).
POOL_DR = True
RL_DT = FP8E4 if POOL_DR else BF16
DR = mybir.MatmulPerfMode.DoubleRow


def _build_pmats():
    """POOL_DR: DoubleRow stationaries for 64-window strips (1536 rows =
    12 pairs), packed [128, 768]: col 128*jp + 64*i + w holds
    P'_{2jp+i}[tau, w] with P'_j[tau, (128*j + tau) // 24] = 1 for
    j = 0..11, w = 0..63 (the strip index cancels: 1536/24 = 64).

    Fallback: serial stationaries for 32-window strips, packed [128, 192]:
    col 64*jp + 32*i + w holds P_{2jp+i}[tau, (128*(2jp+i) + tau)//24]."""
    if POOL_DR:
        pm = np.zeros((128, 768), np.float32)
        for j_ in range(12):
            jp, i = divmod(j_, 2)
            for tau in range(128):
                pm[tau, 128 * jp + 64 * i + (128 * j_ + tau) // 24] = 1.0
        return pm
    pm = np.zeros((128, 192), np.float32)
    for j_ in range(6):
        jp, i = divmod(j_, 2)
        for tau in range(128):
            pm[tau, 64 * jp + 32 * i + (128 * j_ + tau) // 24] = 1.0
    return pm


def _build_w2tp(W2):
    """W2/(24*SX) arranged for matmul2 over 128-row k-chunks:
    w2tp[r, 10c+cc] = W2[cc, 128c+r]/(24*SX) (zero past k=400)."""
    w = np.zeros((128, 40), np.float32)
    for c in range(4):
        k0 = 128 * c
        kn = min(400, k0 + 128) - k0
        w[:kn, 10 * c : 10 * c + 10] = (
            W2.astype(np.float32).T[k0 : k0 + kn]
        ) / (J * SX)
    return w.astype(nbf)


def _build_nc(repeat: int = 1):
    """repeat>1 re-runs the whole computation in one NEFF — used by the
    test harness to measure device time differentially."""
    nc = bass.Bass()
    PMC = 768 if POOL_DR else 192
    xq_d = nc.declare_dram_parameter("xq", [100, H], FP8, isOutput=False)
    w1t_d = nc.declare_dram_parameter("w1t", [50, 400], BF16, isOutput=False)
    w2tp_d = nc.declare_dram_parameter("w2tp", [128, 40], BF16, isOutput=False)
    pm_d = nc.declare_dram_parameter("pmats", [128, PMC], RL_DT, isOutput=False)
    out_d = nc.declare_dram_parameter("logits", [B_CORE, 10], F32, isOutput=True)

    act = mybir.ActivationFunctionType

    with TileContext(nc) as tc:
        with (
            tc.tile_pool(name="consts", bufs=1) as cpool,
            tc.tile_pool(name="xchunks", bufs=3) as xpool,
            tc.tile_pool(name="relu", bufs=19 if POOL_DR else 13) as rpool,
            tc.tile_pool(name="featsb", bufs=2) as fspool,
            tc.tile_pool(name="featT", bufs=2) as ftpool,
            tc.tile_pool(name="lsb", bufs=2) as lpool,
            # POOL_DR: feat accumulator bank 0 + mm1 ring banks 1-6 +
            # logits bank 7. Fallback: mm1 ring 6 banks + feat 2 banks
            # (lps shares the mm1 ring).
            tc.tile_pool(name="featps", bufs=1 if POOL_DR else 2, space="PSUM")
            as featpool,
            tc.tile_pool(name="mm1ps", bufs=3, space="PSUM") as mm1pool,
            tc.tile_pool(name="lps", bufs=1, space="PSUM") as lpspool,
        ):
            # W1T staged at partition offsets 0 and 64 — the moving operand
            # must share the stationary's base partition (array row offset).
            w1t = cpool.tile([128, 400], BF16)
            w2tp = cpool.tile([128, 40], BF16)
            pmats = cpool.tile([128, PMC], RL_DT)
            nc.sync.dma_start(out=w1t[0:50, :], in_=w1t_d[:])
            nc.sync.dma_start(out=w1t[64:114, :], in_=w1t_d[:])
            nc.sync.dma_start(out=w2tp[:], in_=w2tp_d[:])
            nc.sync.dma_start(out=pmats[:], in_=pm_d[:])

            # The single DoubleRow-legal accumulator: allocated first so it
            # lands in psum bank 0 (DR dst must be bank 0, partition base 0).
            feat_acc = (
                featpool.tile([128, 512], F32, name="feat_acc")
                if POOL_DR
                else None
            )

            def emit_dr_pool(pairs, st_, hh, jp):
                """One DoubleRow pooling matmul: windows 64*st_..+63 of
                half hh accumulate P'_{2jp}^T @ rl_{2jp} + P'_{2jp+1}^T @
                rl_{2jp+1} into feat_acc[0:64]."""
                rl2 = pairs[6 * st_ + jp]
                stat = pmats[:, 128 * jp : 128 * jp + 128].rearrange(
                    "p (two w) -> p two w", two=2
                )
                nc.tensor.matmul(
                    feat_acc[0:64, 0:400],
                    stat,
                    rl2[:, :, hh, :],
                    start=(jp == 0),
                    stop=(jp == 5),
                    perf_mode=DR,
                )

            def emit_feat_evac(st, st_, hh):
                """feat_acc -> feat_sb block f = 2*st_ + hh (bf16)."""
                f = 2 * st_ + hh
                if f % 2 == 0:
                    nc.scalar.activation(
                        st["feat_sb"][0:64, f, 0:400],
                        feat_acc[0:64, 0:400],
                        act.Relu,
                    )
                else:
                    nc.vector.tensor_scalar_max(
                        st["feat_sb"][0:64, f, 0:400],
                        feat_acc[0:64, 0:400],
                        0.0,
                    )

            def emit_tail_a(st):
                """xbar transpose of the completed feat_sb (POOL_DR) or
                feat psum evac + transpose (fallback)."""
                if POOL_DR:
                    featT = ftpool.tile([128, 16, 64], BF16, name="ftT")
                    nc.sync.dma_start_transpose(
                        out=featT[:],
                        in_=st["feat_sb"][:].rearrange("p f k -> p (f k)"),
                    )
                else:
                    feat_sb = fspool.tile([128, 2, 512], BF16, name="fsb")
                    nc.gpsimd.memset(feat_sb[:, :, 400:512], 0.0)
                    nc.scalar.activation(
                        feat_sb[:, 0, 0:400], st["featps"][0][:, 0:400], act.Relu
                    )
                    nc.vector.tensor_scalar_max(
                        feat_sb[:, 1, 0:400], st["featps"][1][:, 0:400], 0.0
                    )
                    featT = ftpool.tile([128, 8, 128], BF16, name="ftT")
                    nc.sync.dma_start_transpose(
                        out=featT[:],
                        in_=feat_sb[:].rearrange("p two k -> p (two k)"),
                    )
                st["featT"] = featT

            def emit_tail_b(st):
                """matmul2 over k-chunks + logits psum -> sbuf -> DRAM."""
                featT = st["featT"]
                g = st["G"]
                if POOL_DR:
                    lps = lpspool.tile([128, 512], F32, name="lps")
                    for f in range(4):          # f = 2*st_ + hh
                        for c_ in range(4):
                            nc.tensor.matmul(
                                lps[0:64, 16 * f : 16 * f + 10],
                                featT[:, 4 * f + c_, :],
                                w2tp[:, 10 * c_ : 10 * c_ + 10],
                                start=(c_ == 0),
                                stop=(c_ == 3),
                            )
                    lsb = lpool.tile([64, 4, 10], F32, name="lsb")
                    src = lps[0:64, 0:64].rearrange("p (f k) -> p f k", f=4)[
                        :, :, 0:10
                    ]
                    nc.vector.tensor_copy(out=lsb[:], in_=src)
                    # DRAM row = 2048*hh + 128*g + 64*st_ + p
                    dstv = out_d[:].rearrange(
                        "(hh g st p) k -> hh g p st k", hh=2, g=NG, st=2, p=64
                    )
                    srcv = lsb[:].rearrange(
                        "p (st hh) k -> p st hh k", st=2
                    )
                    for hh in range(2):
                        nc.sync.dma_start(
                            out=dstv[hh, g], in_=srcv[:, :, hh, :]
                        )
                else:
                    lps = mm1pool.tile([128, 1024], F32, name="ps")
                    for hh in range(2):
                        for c_ in range(4):
                            nc.tensor.matmul(
                                lps[:, 16 * hh : 16 * hh + 10],
                                featT[:, 4 * hh + c_, :],
                                w2tp[:, 10 * c_ : 10 * c_ + 10],
                                start=(c_ == 0),
                                stop=(c_ == 3),
                            )
                    lsb = lpool.tile([128, 2, 10], F32, name="lsb")
                    src = lps[:, 0:32].rearrange("p (two k) -> p two k", two=2)[
                        :, :, 0:10
                    ]
                    nc.vector.tensor_copy(out=lsb[:], in_=src)
                    dst = out_d[:].rearrange(
                        "(two w) c -> w two c", two=2
                    )[128 * g : 128 * g + 128]
                    nc.sync.dma_start(out=dst, in_=lsb[:])

            prev = None  # supergroup awaiting tail-a (feat evac + transpose)
            prev2 = None  # supergroup awaiting tail-b (mm2 + store)
            for G in [g for _ in range(repeat) for g in range(NG)]:
                xc = xpool.tile([128, CHUNK], FP8, name="xc")
                nc.sync.dma_start(
                    out=xc[0:50, :], in_=xq_d[0:50, G * CHUNK : (G + 1) * CHUNK]
                )
                nc.sync.dma_start(
                    out=xc[64:114, :], in_=xq_d[50:100, G * CHUNK : (G + 1) * CHUNK]
                )

                if prev is not None:
                    emit_tail_a(prev)

                # ---- matmul1 + relu evacuation (24 pairs, written
                # two-to-a-tile) interleaved with lagged pooling so the
                # evac engines never drain at supergroup boundaries ----
                pairs = []  # 12 tiles [128, 2(pair member), 2(hh), 400]
                if POOL_DR:
                    st = {"G": G}
                    feat_sb = fspool.tile([64, 4, 512], BF16, name="fsb")
                    nc.gpsimd.memset(feat_sb[:, :, 400:512], 0.0)
                    st["feat_sb"] = feat_sb
                    featps = None
                else:
                    st = None
                    featps = [
                        featpool.tile([128, 512], F32, name="featps")
                        for _ in range(2)
                    ]

                def emit_pool(ip):
                    """Fallback serial pooling of pair-group ip into strip
                    s = ip//3 (32-window strips)."""
                    rl2 = pairs[ip]
                    s, jp = ip // 3, ip % 3
                    for im in range(2):
                        j_ = 2 * jp + im
                        for hh in range(2):
                            nc.tensor.matmul(
                                featps[hh][32 * s : 32 * s + 32, 0:400],
                                pmats[
                                    :,
                                    64 * jp + 32 * im : 64 * jp + 32 * im + 32,
                                ],
                                rl2[:, im, hh, :],
                                start=(j_ == 0),
                                stop=(j_ == 5),
                                tile_position=(0, 32 * s),
                            )

                for ip in range(NP // 2):
                    rl2 = rpool.tile(
                        [128, 2, 2, 400], RL_DT, name="rl",
                        bufs=19 if POOL_DR else 13,
                    )
                    for im in range(2):
                        i = 2 * ip + im
                        tcol = i * 128
                        ps = mm1pool.tile([128, 1024], F32, name="ps")
                        for hh in range(2):
                            rb = 64 * hh
                            nc.tensor.matmul(
                                ps[:, 512 * hh : 512 * hh + 400],
                                xc[rb : rb + 50, tcol : tcol + 128],
                                w1t[rb : rb + 50, :],
                                start=True,
                                stop=True,
                            )
                        src = ps[:, :].rearrange("p (two k) -> p two k", two=2)[
                            :, :, 0:400
                        ]
                        # ACT ~850ns vs DVE ~960ns per pair (PSUM-sourced);
                        # ACT also owns half of tail-a -> 13:11 split
                        if im == 0 or ip == 5:
                            nc.scalar.activation(rl2[:, im, :, :], src, act.Relu)
                        else:
                            nc.vector.tensor_scalar_max(rl2[:, im, :, :], src, 0.0)
                    pairs.append(rl2)
                    if POOL_DR:
                        # round (st_=0, hh=0): DRs lag 2 groups; round
                        # (st_=1, hh=0) starts a group after its evac; the
                        # hh=1 rounds trail (tile ring has 19 bufs so sg
                        # G+1's groups don't clobber tiles 0-5 meanwhile).
                        if 2 <= ip <= 7:
                            emit_dr_pool(pairs, 0, 0, ip - 2)
                        if ip == 8:
                            emit_feat_evac(st, 0, 0)
                        if 9 <= ip <= 11:
                            emit_dr_pool(pairs, 1, 0, ip - 9)
                    else:
                        if ip >= 2:
                            emit_pool(ip - 2)
                    if ip == 5 and prev2 is not None:
                        emit_tail_b(prev2)

                if POOL_DR:
                    for jp in (3, 4, 5):
                        emit_dr_pool(pairs, 1, 0, jp)
                    emit_feat_evac(st, 1, 0)
                    for jp in range(6):
                        emit_dr_pool(pairs, 0, 1, jp)
                    emit_feat_evac(st, 0, 1)
                    for jp in range(6):
                        emit_dr_pool(pairs, 1, 1, jp)
                    emit_feat_evac(st, 1, 1)
                    prev2 = prev
                    prev = st
                else:
                    emit_pool(10)
                    emit_pool(11)
                    prev2 = prev
                    prev = {"G": G, "featps": featps}

            # drain the tail pipeline
            if prev is not None:
                emit_tail_a(prev)
            if prev2 is not None:
                emit_tail_b(prev2)
            emit_tail_b(prev)
    return nc


_NC = {}


def _get_nc(repeat: int = 1):
    if repeat not in _NC:
        _NC[repeat] = _build_nc(repeat)
    return _NC[repeat]


def prepare_in_maps(x: np.ndarray, W1: np.ndarray, W2: np.ndarray):
    assert x.shape == (T, N) and W1.shape == (K, N) and W2.shape == (C, K)

    w1t = np.ascontiguousarray(W1.T.astype(nbf))          # [50, 400]
    w2tp = _build_w2tp(W2)                                 # [128, 40]
    pmats = _build_pmats().astype(mybir.dt.np(RL_DT))

    xb = np.clip(x.astype(np.float32) * SX, -15.0, 15.0).astype(nf8)
    in_maps = []
    for c in range(NCORES):
        shard = xb[c * TC : (c + 1) * TC]                  # [98304, 50]
        xq = np.empty((100, H), nf8)
        xq[0:50] = shard[0:H].T
        xq[50:100] = shard[H:].T
        in_maps.append(
            {
                "xq": xq,
                "w1t": w1t,
                "w2tp": w2tp,
                "pmats": pmats,
            }
        )
    return in_maps


def kernel(x: np.ndarray, W1: np.ndarray, W2: np.ndarray) -> np.ndarray:
    in_maps = prepare_in_maps(x, W1, W2)
    nc = _get_nc()
    res = run_bass_kernel_spmd(nc, in_maps, core_ids=list(range(NCORES)))
    out = np.concatenate(
        [res.results[c]["logits"] for c in range(NCORES)], axis=0
    )
    return out.astype(np.float32)
